# revision 4
# baseline (speedup 1.0000x reference)
"""GCNCombiner Trainium2 kernel — 8-core batch-parallel Bass/Tile implementation.

Math (reference):
  hs0 = x_flat @ w_pool0.T + b_pool0          (B, PS, NJ)
  q1  = mean_o(w_q @ hs0 + b_q) = u_q . hs0 + mean(b_q)   (B, NJ)
  k1  likewise
  A1  = adj1 + tanh(q1[:,None] - k1[None,:]) * alpha      (B, NJ, NJ)
  hs1 = w_c1 @ hs0 + b_c1                     (B, PS, NJ)
  hs2 = hs1 @ A1                              (B, PS, NJ)
  BN over (b, j) per channel; pool with w_pool1; classifier.

Because BN is a per-channel affine map s*h+t, the final output only needs
  r[b,c]    = sum_j hs2[b,c,j] * w_pool1[j]
  ssum[c]   = sum_{b,j} hs2[b,c,j]
  ssq[c]    = sum_{b,j} hs2[b,c,j]^2
Each core computes these for its 4 batches; the 8-way reduction of
ssum/ssq (the BN batch-stats all-reduce) and the tiny (32x1536)@(1536x200)
classifier run on the host during the gather/unshard step.

Device schedule: a list-scheduler weaves every batch's PE-light tail
(transposes, q/k, A1, conv1, hs2, stats) into the NEXT batch's
DMA-paced pool0 k-loop, so the PE never drains while x streams.  A
parametric DMA-landing model paces the emission so the in-order engine
queues never block on un-landed data while ready work waits behind; a
per-thunk ready time additionally gates conv1 on its wc1T slab.
Window 0 (no prior tail) is filled with dummy matmuls that hold the
PE p-state/clock at full speed.  The last batch's x is shipped
column-block-major so its own tail can start before pool0 finishes.
x, w_pool0.T and w_c1.T are host-swizzled so every SBUF partition's
bytes are one contiguous DRAM run (12-16KB DMA descriptors).

PSUM budget (16KB/partition): pool0 runs two k-major groups (mpA/mpB,
4KB) and finishes the third column block as a post-landing pass that
reuses mpA; conv1 holds 3 groups (6KB) whose ring also serves hs2 and
the window-0 dummies; transposes/aux/stats rings fill the rest.
"""

import numpy as np

import concourse.bacc as bacc
import concourse.mybir as mybir
import concourse.tile as tile
from concourse.bass_utils import run_bass_kernel_spmd

# problem shapes (hardcoded per contract)
B, PS, H, W = 32, 1536, 32, 64
S = H * W                # 2048 selects
NJ = 128                 # joints
QK = PS // 4
NC = 200
BN_EPS = 1e-5

NCORES = 8
PB = B // NCORES         # batches per core = 4
SK = S // 128            # 16 s-chunks
CK = PS // 128           # 12 c-chunks
NK = PS // 512           # 3 free-dim chunks of 512

F16 = mybir.dt.float16
F32 = mybir.dt.float32
AF = mybir.ActivationFunctionType

TRACE = False            # set True (e.g. from test.py) to profile via NTFF
LAST_EXEC_NS = None
TMPDIR = None
_CACHE = {}

# ---- emission pacing model (ns) -------------------------------------------
DMA_BPNS = 0.33          # ~330 GB/s assumed effective HBM rate
DMA_T0 = 8000.0          # preamble before first descriptor data lands
C_MM512 = 220.0          # 128x128x512 matmul
C_T = 100.0              # 128x128 transpose
C_KQ = 105.0             # kq accumulate step
C_ST = 250.0             # stats matmul
C_AUX = 110.0            # pqt/pbc


def _build_nc(with_bc1=True):
    nc = bacc.Bacc("TRN2", target_bir_lowering=False, debug=False,
                   num_devices=NCORES)

    d = {}
    d["xh"] = nc.dram_tensor("xh", [PB, 128, SK * PS], F16,
                             kind="ExternalInput").ap()
    d["pT"] = nc.dram_tensor("pT", [128, SK * NJ], F16, kind="ExternalInput").ap()
    d["wc1T"] = nc.dram_tensor("wc1T", [128, CK * PS], F16,
                               kind="ExternalInput").ap()
    d["ukq"] = nc.dram_tensor("ukq", [128, CK * 2], F16, kind="ExternalInput").ap()
    d["onesw1"] = nc.dram_tensor("onesw1", [128, 2], F16, kind="ExternalInput").ap()
    d["adj"] = nc.dram_tensor("adj", [NJ, NJ], F32, kind="ExternalInput").ap()
    d["ident"] = nc.dram_tensor("ident", [128, 128], F16, kind="ExternalInput").ap()
    d["ident2"] = nc.dram_tensor("ident2", [2, 2], F32, kind="ExternalInput").ap()
    d["ones1_16"] = nc.dram_tensor("ones1_16", [1, 128], F16, kind="ExternalInput").ap()
    d["ones1_32"] = nc.dram_tensor("ones1_32", [1, 128], F32, kind="ExternalInput").ap()
    d["bc1"] = nc.dram_tensor("bc1", [1, PS], F16, kind="ExternalInput").ap()
    d["bp0"] = nc.dram_tensor("bp0", [128, 1], F32, kind="ExternalInput").ap()
    d["bkq"] = nc.dram_tensor("bkq", [2, 1], F32, kind="ExternalInput").ap()
    d["alphac"] = nc.dram_tensor("alphac", [128, 1], F32, kind="ExternalInput").ap()

    # per batch: [ssum, r, ssq] concatenated along the free dim
    rss_out = nc.dram_tensor("rss_out", [PB, 3, PS], F32,
                             kind="ExternalOutput").ap()

    QB = SK * PS // 4        # x quarter, free elems (4 k-chunks)
    TB = SK * PS // 3        # x third for the n-major last batch
    WS = CK * PS // 3        # wc1T slab

    with tile.TileContext(nc) as tc:
        with tc.tile_pool(name="const", bufs=1) as cp, \
             tc.tile_pool(name="xp", bufs=2) as xp, \
             tc.tile_pool(name="work", bufs=2) as wp, \
             tc.tile_pool(name="sm", bufs=2) as smp, \
             tc.tile_pool(name="rp", bufs=2) as rp, \
             tc.tile_pool(name="mp0", bufs=1, space="PSUM") as pp0, \
             tc.tile_pool(name="mch", bufs=3, space="PSUM") as pch, \
             tc.tile_pool(name="sml", bufs=2, space="PSUM") as psml, \
             tc.tile_pool(name="auk", bufs=1, space="PSUM") as pakq:

            # ---- DMA issue order fixes the landing schedule:
            # pT, x0 (4 quarters), x1q0, x1q1, wc1s0, x1q2, wc1s1, x1q3,
            # wc1s2, x2 (4 quarters), x3 (3 thirds) ----
            pT_sb = cp.tile([128, SK * NJ], F16, tag="pT")
            nc.sync.dma_start(out=pT_sb[:], in_=d["pT"])

            x_sb = [None] * PB
            wc1_sb = cp.tile([128, CK * PS], F16, tag="wc1")

            def x_tile(b):
                x_sb[b] = xp.tile([128, SK * PS], F16, tag="x",
                                  name=f"x_sb{b}")

            def x_quarter(b, qi):
                nc.sync.dma_start(out=x_sb[b][:, qi * QB:(qi + 1) * QB],
                                  in_=d["xh"][b, :, qi * QB:(qi + 1) * QB])

            def x_third(b, ti):
                nc.sync.dma_start(out=x_sb[b][:, ti * TB:(ti + 1) * TB],
                                  in_=d["xh"][b, :, ti * TB:(ti + 1) * TB])

            def wc1_slab(si):
                nc.sync.dma_start(out=wc1_sb[:, si * WS:(si + 1) * WS],
                                  in_=d["wc1T"][:, si * WS:(si + 1) * WS])

            x_tile(0)
            for qi in range(4):
                x_quarter(0, qi)
            x_tile(1)
            x_quarter(1, 0)
            x_quarter(1, 1)
            wc1_slab(0)
            x_quarter(1, 2)
            wc1_slab(1)
            x_quarter(1, 3)
            wc1_slab(2)
            x_tile(2)
            for qi in range(4):
                x_quarter(2, qi)
            x_tile(3)
            for ti in range(3):
                x_third(3, ti)

            # small constants ride the gpsimd (SWDGE) queue in parallel
            ukq_sb = cp.tile([128, CK * 2], F16, tag="ukq")
            nc.gpsimd.dma_start(out=ukq_sb[:], in_=d["ukq"])
            onesw1_sb = cp.tile([128, 2], F16, tag="onesw1")
            nc.gpsimd.dma_start(out=onesw1_sb[:], in_=d["onesw1"])
            adj_sb = cp.tile([NJ, NJ], F32, tag="adj")
            nc.gpsimd.dma_start(out=adj_sb[:], in_=d["adj"])
            ident_sb = cp.tile([128, 128], F16, tag="ident")
            nc.gpsimd.dma_start(out=ident_sb[:], in_=d["ident"])
            ident2_sb = cp.tile([2, 2], F32, tag="ident2")
            nc.gpsimd.dma_start(out=ident2_sb[:], in_=d["ident2"])
            ones16_sb = cp.tile([1, 128], F16, tag="ones16")
            nc.gpsimd.dma_start(out=ones16_sb[:], in_=d["ones1_16"])
            ones32_sb = cp.tile([1, 128], F32, tag="ones32")
            nc.gpsimd.dma_start(out=ones32_sb[:], in_=d["ones1_32"])
            bc1_sb = cp.tile([1, PS], F16, tag="bc1")
            nc.gpsimd.dma_start(out=bc1_sb[:], in_=d["bc1"])
            bp0_sb = cp.tile([128, 1], F32, tag="bp0")
            nc.gpsimd.dma_start(out=bp0_sb[:], in_=d["bp0"])
            bkq_sb = cp.tile([2, 1], F32, tag="bkq")
            nc.gpsimd.dma_start(out=bkq_sb[:], in_=d["bkq"])
            alpha_sb = cp.tile([128, 1], F32, tag="alphac")
            nc.gpsimd.dma_start(out=alpha_sb[:], in_=d["alphac"])

            # dummy-matmul tile for PE p-state keepalive in window 0
            wu_sb = cp.tile([128, 512], F16, tag="wu")
            nc.vector.memset(wu_sb[:], 0.0)
            wu_ct = [0]

            def emit_wu():
                pw = pch.tile([128, 512], F32, tag="mmt",
                              name=f"wu{wu_ct[0]}")
                wu_ct[0] += 1
                nc.tensor.matmul(pw[:], wu_sb[:, 0:128], wu_sb[:],
                                 start=True, stop=True)

            # ---- per-batch state ----
            hs0T = [[None] * NK for _ in range(PB)]
            hs0 = [None] * PB       # [128, CK*NJ] f16 (c-partition layout)
            hs1T = [None] * PB      # [128, PS] f16
            a1 = [None] * PB        # [NJ, NJ] f16
            kq = [None] * PB
            h2cs = [[None] * NK for _ in range(PB)]
            sqcs = [[None] * NK for _ in range(PB)]
            psAB = [None] * PB      # pool0 k-major psum pair
            psC = [None] * PB       # pool0 pass-B psum
            pcs = [None] * PB       # conv1 psum tiles
            pkq = [None] * PB
            sr_sb = [None] * PB
            ssq_sb = [None] * PB

            # ---------------- emission pieces -------------------------------
            def pool0_stepAB(b, k):
                """One pool0 k-step over column blocks 0,1 (k-major x)."""
                for n in range(2):
                    nc.tensor.matmul(
                        psAB[b][n][:],
                        pT_sb[:, k * NJ:(k + 1) * NJ],
                        x_sb[b][:, k * PS + n * 512: k * PS + n * 512 + 512],
                        start=(k == 0), stop=(k == SK - 1))
                if k == SK - 1:
                    for n in range(2):
                        nc.vector.tensor_scalar_add(
                            hs0T[b][n][:], psAB[b][n][:], bp0_sb[:])

            def passB_group(b, g):
                """Pool0 column block 2, 4 k-steps per thunk (x landed)."""
                if g == 0:
                    psC[b] = pp0.tile([128, 512], F32, tag="mpA",
                                      name=f"pC{b}")
                for k in range(4 * g, 4 * g + 4):
                    nc.tensor.matmul(
                        psC[b][:],
                        pT_sb[:, k * NJ:(k + 1) * NJ],
                        x_sb[b][:, k * PS + 1024: k * PS + 1536],
                        start=(k == 0), stop=(k == SK - 1))
                if g == 3:
                    nc.vector.tensor_scalar_add(
                        hs0T[b][2][:], psC[b][:], bp0_sb[:])

            def pool0_nstep(b, n, k):
                """Last batch (column-block-major x): one (n, k) matmul."""
                if k == 0:
                    psC[b] = pp0.tile([128, 512], F32, tag="mpA",
                                      name=f"pL{b}_{n}")
                nc.tensor.matmul(
                    psC[b][:],
                    pT_sb[:, k * NJ:(k + 1) * NJ],
                    x_sb[b][:, n * (SK * 512) + k * 512:
                             n * (SK * 512) + k * 512 + 512],
                    start=(k == 0), stop=(k == SK - 1))
                if k == SK - 1:
                    nc.vector.tensor_scalar_add(
                        hs0T[b][n][:], psC[b][:], bp0_sb[:])

            def emit_T(b, cc):
                pt = psml.tile([128, 128], F16, tag="sml", name=f"tr{b}_{cc}")
                nc.tensor.transpose(
                    pt[:],
                    hs0T[b][cc // 4][:, (cc % 4) * 128:(cc % 4) * 128 + 128],
                    ident_sb[:])
                nc.vector.tensor_copy(hs0[b][:, cc * NJ:(cc + 1) * NJ], pt[:])

            def emit_kq(b, cc):
                if cc == 0:
                    pkq[b] = pakq.tile([2, 128], F32, tag="kqt",
                                       name=f"pkq{b}")
                nc.tensor.matmul(pkq[b][:], ukq_sb[:, 2 * cc:2 * cc + 2],
                                 hs0[b][:, cc * NJ:(cc + 1) * NJ],
                                 start=(cc == 0), stop=(cc == CK - 1))
                if cc == CK - 1:
                    kq[b] = smp.tile([2, 128], F32, tag="kq", name=f"kq{b}")
                    nc.scalar.activation(kq[b][:], pkq[b][:], AF.Identity,
                                         bias=bkq_sb[:])

            def emit_a1(b):
                """A1 = adj + alpha * tanh(q[j] - k[j']): two PE ops + DVE."""
                pqt = psml.tile([128, 2], F32, tag="sml", name=f"pqt{b}")
                nc.tensor.transpose(pqt[:], kq[b][:], ident2_sb[:])
                qcol_sb = smp.tile([128, 1], F32, tag="qcol", name=f"qcol{b}")
                nc.scalar.activation(qcol_sb[:], pqt[:, 1:2], AF.Copy)
                pbc = psml.tile([128, 128], F32, tag="sml", name=f"pbc{b}")
                nc.tensor.matmul(pbc[:], ones32_sb[:], kq[b][0:1, :],
                                 start=True, stop=True)
                tanh_sb = smp.tile([128, 128], F32, tag="tanh",
                                   name=f"tanh{b}")
                nc.scalar.activation(tanh_sb[:], pbc[:], AF.Tanh,
                                     scale=-1.0, bias=qcol_sb[:])
                a1[b] = smp.tile([NJ, NJ], F16, tag="a1", name=f"a1_{b}")
                nc.vector.tensor_scalar_mul(tanh_sb[:], tanh_sb[:],
                                            alpha_sb[:])
                nc.vector.tensor_add(a1[b][:], tanh_sb[:], adj_sb[:])

            def emit_conv1(b, cc):
                if cc == 0:
                    pcs[b] = [pch.tile([128, 512], F32, tag="mmt",
                                       name=f"c1_{b}_{n}") for n in range(NK)]
                last = cc == CK - 1
                for n in range(NK):
                    nc.tensor.matmul(
                        pcs[b][n][:],
                        hs0[b][:, cc * NJ:(cc + 1) * NJ],
                        wc1_sb[:, cc * PS + n * 512: cc * PS + n * 512 + 512],
                        start=(cc == 0),
                        stop=(not with_bc1 and last))
                if last:
                    if with_bc1:
                        for n in range(NK):
                            nc.tensor.matmul(pcs[b][n][:], ones16_sb[:],
                                             bc1_sb[:, n * 512:(n + 1) * 512],
                                             start=False, stop=True)
                    for n in range(NK):
                        nc.scalar.activation(
                            hs1T[b][:, n * 512:(n + 1) * 512],
                            pcs[b][n][:], AF.Copy)

            def emit_h2(b, n):
                ph = pch.tile([128, 512], F32, tag="mmt", name=f"h2_{b}_{n}")
                nc.tensor.matmul(ph[:], a1[b][:],
                                 hs1T[b][:, n * 512:(n + 1) * 512],
                                 start=True, stop=True)
                h2_sb = wp.tile([128, 512], F16, tag=f"h2c{n}",
                                name=f"h2c{b}_{n}")
                sq_sb = wp.tile([128, 512], F16, tag=f"sqc{n}",
                                name=f"sqc{b}_{n}")
                nc.vector.tensor_copy(h2_sb[:], ph[:])
                nc.scalar.activation(sq_sb[:], ph[:], AF.Square)
                h2cs[b][n] = h2_sb
                sqcs[b][n] = sq_sb

            def emit_stats(b, n):
                if n == 0:
                    sr_sb[b] = rp.tile([2, PS], F32, tag="sr", name=f"sr{b}")
                    ssq_sb[b] = rp.tile([1, PS], F32, tag="ssq",
                                        name=f"ssq{b}")
                sl = slice(n * 512, (n + 1) * 512)
                prs = psml.tile([2, 512], F32, tag="sml", name=f"prs{b}_{n}")
                nc.tensor.matmul(prs[:], onesw1_sb[:], h2cs[b][n][:],
                                 start=True, stop=True)
                nc.scalar.activation(sr_sb[b][:, sl], prs[:], AF.Copy)
                pq2 = psml.tile([1, 512], F32, tag="sml", name=f"pq2{b}_{n}")
                nc.tensor.matmul(pq2[:], onesw1_sb[:, 0:1], sqcs[b][n][:],
                                 start=True, stop=True)
                nc.scalar.activation(ssq_sb[b][:, sl], pq2[:], AF.Copy)
                if n == NK - 1:
                    # last batch rides the HWDGE ring: lower completion
                    # latency on the kernel's critical tail
                    eng = nc.sync if b == PB - 1 else nc.gpsimd
                    eng.dma_start(out=rss_out[b, 0:2, :], in_=sr_sb[b][:])
                    eng.dma_start(out=rss_out[b, 2:3, :], in_=ssq_sb[b][:])

            # ---------------- landing-time model ----------------------------
            pT_B = 128 * SK * NJ * 2
            xq_B = 128 * QB * 2
            xt_B = 128 * TB * 2
            wcs_B = 128 * WS * 2

            land_x = [[0.0] * 4 for _ in range(PB)]
            land_wc = [0.0] * 3
            cum = [pT_B]

            def land(nbytes):
                cum[0] += nbytes
                return DMA_T0 + cum[0] / DMA_BPNS

            for qi in range(4):
                land_x[0][qi] = land(xq_B)
            land_x[1][0] = land(xq_B)
            land_x[1][1] = land(xq_B)
            land_wc[0] = land(wcs_B)
            land_x[1][2] = land(xq_B)
            land_wc[1] = land(wcs_B)
            land_x[1][3] = land(xq_B)
            land_wc[2] = land(wcs_B)
            for qi in range(4):
                land_x[2][qi] = land(xq_B)
            for ti in range(3):
                land_x[3][ti] = land(xt_B)

            # ---------------- thunk lists -----------------------------------
            def passB_thunks(b):
                return [(4 * C_MM512, land_x[b][3] + 300.0,
                         lambda b=b, g=g: passB_group(b, g))
                        for g in range(4)]

            def tail_thunks(b):
                th = []
                for cc in range(CK):
                    th.append((C_T, 0.0, lambda b=b, cc=cc: emit_T(b, cc)))
                for cc in range(CK):
                    th.append((C_KQ, 0.0, lambda b=b, cc=cc: emit_kq(b, cc)))
                th.append((2 * C_AUX, 0.0, lambda b=b: emit_a1(b)))
                for cc in range(CK):
                    th.append((NK * C_MM512, land_wc[cc // 4] + 300.0,
                               lambda b=b, cc=cc: emit_conv1(b, cc)))
                return th

            def late_thunks(b):
                th = []
                for n in range(NK):
                    th.append((C_MM512, 0.0, lambda b=b, n=n: emit_h2(b, n)))
                for n in range(NK):
                    th.append((2 * C_ST, 0.0,
                               lambda b=b, n=n: emit_stats(b, n)))
                return th

            t_pe = [2000.0]

            def emit_tail(queue, upto):
                while queue and t_pe[0] < upto and queue[0][1] <= t_pe[0]:
                    c, rdy, fn = queue.pop(0)
                    fn()
                    t_pe[0] += c

            # ---------------- main emission loop ----------------------------
            queue = []
            for b in range(PB):
                hs0T[b] = [wp.tile([128, 512], F16, tag=f"hs0T{n}",
                                   name=f"hs0T{b}_{n}") for n in range(NK)]
                hs0[b] = wp.tile([128, CK * NJ], F16, tag="hs0",
                                 name=f"hs0_{b}")
                hs1T[b] = wp.tile([128, PS], F16, tag="hs1T", name=f"hs1T{b}")

                if b >= 1:
                    queue.extend(passB_thunks(b - 1))
                if b >= 2:
                    queue.extend(late_thunks(b - 2))
                if b >= 1:
                    queue.extend(tail_thunks(b - 1))

                if b < PB - 1:
                    psAB[b] = [pp0.tile([128, 512], F32, tag=t,
                                        name=f"p0_{b}_{t}")
                               for t in ("mpA", "mpB")]
                    for k in range(SK):
                        need = land_x[b][k // 4] + 300.0
                        if queue:
                            emit_tail(queue, need)
                        if b == 0:
                            while t_pe[0] < need - C_MM512:
                                emit_wu()
                                t_pe[0] += C_MM512
                        if t_pe[0] < need:
                            t_pe[0] = need
                        pool0_stepAB(b, k)
                        t_pe[0] += 2 * C_MM512
                else:
                    for n in range(NK):
                        for k in range(SK):
                            need = land_x[b][n] + 300.0
                            if queue:
                                emit_tail(queue, need)
                            if t_pe[0] < need:
                                t_pe[0] = need
                            pool0_nstep(b, n, k)
                            t_pe[0] += C_MM512
                        queue.extend(
                            [(C_T, 0.0, lambda b=b, cc=cc: emit_T(b, cc))
                             for cc in range(4 * n, 4 * n + 4)])
                    for cc in range(CK):
                        queue.append(
                            (C_KQ, 0.0, lambda b=b, cc=cc: emit_kq(b, cc)))
                    queue.append((2 * C_AUX, 0.0, lambda b=b: emit_a1(b)))
                    for cc in range(CK):
                        queue.append((NK * C_MM512, 0.0,
                                      lambda b=b, cc=cc: emit_conv1(b, cc)))

            # drain: leftovers, then the last two batches' hs2 + stats
            for c, rdy, fn in queue:
                fn()
            for c, rdy, fn in late_thunks(PB - 2):
                fn()
            for c, rdy, fn in late_thunks(PB - 1):
                fn()

    nc.compile()
    return nc


def _get_nc(with_bc1):
    key = ("nc", with_bc1)
    if key not in _CACHE:
        _CACHE[key] = _build_nc(with_bc1)
    return _CACHE[key]


def kernel(x, w_pool0, b_pool0, adj1, w_q, b_q, w_k, b_k, alpha,
           w_c1, b_c1, gamma, beta, w_pool1, b_pool1, w_cls, b_cls):
    global LAST_EXEC_NS
    x = np.asarray(x, np.float32)

    # ---- host-side input prep (sharding + weight folding) ----
    # (B, S, PS) transpose, then partition-major swizzle: row p holds
    # [xT[k*128+p, :] for k in range(SK)] concatenated.  The last batch of
    # each core instead holds [xT[k*128+p, n*512:(n+1)*512] for n, for k]
    # (column-block-major) so pool0 can run n-outer as thirds land.
    xt = x.reshape(B, PS, S).transpose(0, 2, 1).astype(np.float16)
    xh = np.ascontiguousarray(
        xt.reshape(B, SK, 128, PS).transpose(0, 2, 1, 3)).reshape(
        B, 128, SK * PS)
    pT = np.ascontiguousarray(np.asarray(w_pool0, np.float32).T).astype(np.float16)
    u_q = (np.asarray(w_q, np.float32).sum(0) / QK)
    u_k = (np.asarray(w_k, np.float32).sum(0) / QK)
    ukq = np.stack([u_k, u_q], 1).astype(np.float16)                # (PS, 2)
    wc1T = np.ascontiguousarray(np.asarray(w_c1, np.float32).T).astype(np.float16)
    onesw1 = np.stack([np.ones(NJ, np.float32),
                       np.asarray(w_pool1, np.float32)[0]], 1).astype(np.float16)

    common = {
        "pT": np.ascontiguousarray(
            pT.reshape(SK, 128, NJ).transpose(1, 0, 2)).reshape(128, SK * NJ),
        "wc1T": np.ascontiguousarray(
            wc1T.reshape(CK, 128, PS).transpose(1, 0, 2)).reshape(128, CK * PS),
        "ukq": np.ascontiguousarray(
            ukq.reshape(CK, 128, 2).transpose(1, 0, 2)).reshape(128, CK * 2),
        "onesw1": onesw1,
        "adj": np.asarray(adj1, np.float32),
        "ident": np.eye(128, dtype=np.float16),
        "ident2": np.eye(2, dtype=np.float32),
        "ones1_16": np.ones((1, 128), np.float16),
        "ones1_32": np.ones((1, 128), np.float32),
        "bc1": np.asarray(b_c1, np.float32)[None, :].astype(np.float16),
        "bp0": np.asarray(b_pool0, np.float32)[:, None],
        "bkq": np.array([[np.asarray(b_k, np.float32).mean()],
                         [np.asarray(b_q, np.float32).mean()]], np.float32),
        "alphac": np.full((128, 1), np.asarray(alpha, np.float32)[0], np.float32),
    }
    # last batch of each core: column-block-major swizzle
    xh_nmaj = np.ascontiguousarray(
        xt.reshape(B, SK, 128, NK, 512).transpose(0, 2, 3, 1, 4)).reshape(
        B, 128, SK * PS)
    in_maps = []
    for c in range(NCORES):
        m = dict(common)
        xs = np.empty((PB, 128, SK * PS), np.float16)
        xs[:PB - 1] = xh[c * PB:c * PB + PB - 1]
        xs[PB - 1] = xh_nmaj[c * PB + PB - 1]
        m["xh"] = np.ascontiguousarray(xs)
        in_maps.append(m)

    nc = _get_nc(bool(np.any(np.asarray(b_c1))))
    res = run_bass_kernel_spmd(nc, in_maps, list(range(NCORES)), trace=TRACE,
                               tmpdir=TMPDIR)
    LAST_EXEC_NS = res.exec_time_ns

    # ---- host epilogue: BN stats all-reduce + affine + classifier ----
    rss = np.stack([res.results[c]["rss_out"] for c in range(NCORES)])
    ssum = rss[:, :, 0, :].sum((0, 1)).astype(np.float64)
    r_all = rss[:, :, 1, :].reshape(B, PS)
    ssq = rss[:, :, 2, :].sum((0, 1)).astype(np.float64)
    n = B * NJ
    mean = ssum / n
    var = ssq / n - mean * mean
    s = np.asarray(gamma, np.float64) / np.sqrt(var + BN_EPS)
    t = np.asarray(beta, np.float64) - s * mean
    w1sum = float(np.asarray(w_pool1, np.float64)[0].sum())
    pooled = s[None, :] * r_all.astype(np.float64) \
        + (t * w1sum + float(np.asarray(b_pool1)[0]))[None, :]
    out = pooled @ np.asarray(w_cls, np.float64).T + np.asarray(b_cls, np.float64)
    return out.astype(np.float32)


# revision 8
# speedup vs baseline: 135.7917x; 135.7917x over previous
"""GCNCombiner Trainium2 kernel — 8-core batch-parallel Bass/Tile implementation.

Math (reference):
  hs0 = x_flat @ w_pool0.T + b_pool0          (B, PS, NJ)
  q1  = mean_o(w_q @ hs0 + b_q) = u_q . hs0 + mean(b_q)   (B, NJ)
  k1  likewise
  A1  = adj1 + tanh(q1[:,None] - k1[None,:]) * alpha      (B, NJ, NJ)
  hs1 = w_c1 @ hs0 + b_c1                     (B, PS, NJ)
  hs2 = hs1 @ A1                              (B, PS, NJ)
  BN over (b, j) per channel; pool with w_pool1; classifier.

Because BN is a per-channel affine map s*h+t, the final output only needs
  r[b,c]    = sum_j hs2[b,c,j] * w_pool1[j]
  ssum[c]   = sum_{b,j} hs2[b,c,j]
  ssq[c]    = sum_{b,j} hs2[b,c,j]^2
Each core computes these for its 4 batches; the 8-way reduction of
ssum/ssq (the BN batch-stats all-reduce) and the tiny (32x1536)@(1536x200)
classifier run on the host during the gather/unshard step.

Device schedule: a list-scheduler weaves every batch's PE-light tail
(transposes, q/k, A1, conv1, hs2, stats) into the NEXT batch's
DMA-paced pool0 k-loop, so the PE never drains while x streams.  A
parametric DMA-landing model paces the emission so the in-order engine
queues never block on un-landed data while ready work waits behind; a
per-thunk ready time additionally gates conv1 on its wc1T slab.
Window 0 (no prior tail) is filled with dummy matmuls that hold the
PE p-state/clock at full speed.  The last batch's x is shipped
column-block-major so its own tail can start before pool0 finishes.
x, w_pool0.T and w_c1.T are host-swizzled so every SBUF partition's
bytes are one contiguous DRAM run (12-16KB DMA descriptors).

PSUM budget (16KB/partition): pool0 runs two k-major groups (mpA/mpB,
4KB) and finishes the third column block as a post-landing pass that
reuses mpA; conv1 holds 3 groups (6KB) whose ring also serves hs2 and
the window-0 dummies; transposes/aux/stats rings fill the rest.
"""

import numpy as np

import concourse.bacc as bacc
import concourse.mybir as mybir
import concourse.tile as tile
from concourse.bass_utils import run_bass_kernel_spmd

# problem shapes (hardcoded per contract)
B, PS, H, W = 32, 1536, 32, 64
S = H * W                # 2048 selects
NJ = 128                 # joints
QK = PS // 4
NC = 200
BN_EPS = 1e-5

NCORES = 8
PB = B // NCORES         # batches per core = 4
SK = S // 128            # 16 s-chunks
CK = PS // 128           # 12 c-chunks
NK = PS // 512           # 3 free-dim chunks of 512

F16 = mybir.dt.float16
F32 = mybir.dt.float32
AF = mybir.ActivationFunctionType

TRACE = False            # set True (e.g. from test.py) to profile via NTFF
LAST_EXEC_NS = None
TMPDIR = None
_CACHE = {}

# ---- emission pacing model (ns) -------------------------------------------
DMA_BPNS = 330.0         # ~330 GB/s assumed effective HBM rate (bytes/ns)
DMA_T0 = 8000.0          # preamble before first descriptor data lands
C_MM512 = 220.0          # 128x128x512 matmul
C_T = 100.0              # 128x128 transpose
C_KQ = 105.0             # kq accumulate step
C_ST = 250.0             # stats matmul
C_AUX = 110.0            # pqt/pbc


def _build_nc(with_bc1=True):
    nc = bacc.Bacc("TRN2", target_bir_lowering=False, debug=False,
                   num_devices=NCORES)

    d = {}
    d["xh"] = nc.dram_tensor("xh", [PB, 128, SK * PS], F16,
                             kind="ExternalInput").ap()
    d["pT"] = nc.dram_tensor("pT", [128, SK * NJ], F16, kind="ExternalInput").ap()
    d["wc1T"] = nc.dram_tensor("wc1T", [128, CK * PS], F16,
                               kind="ExternalInput").ap()
    d["ukq"] = nc.dram_tensor("ukq", [128, CK * 2], F16, kind="ExternalInput").ap()
    d["onesw1"] = nc.dram_tensor("onesw1", [128, 2], F16, kind="ExternalInput").ap()
    d["adj"] = nc.dram_tensor("adj", [NJ, NJ], F32, kind="ExternalInput").ap()
    d["ident"] = nc.dram_tensor("ident", [128, 128], F16, kind="ExternalInput").ap()
    d["ident2"] = nc.dram_tensor("ident2", [2, 2], F32, kind="ExternalInput").ap()
    d["ones1_16"] = nc.dram_tensor("ones1_16", [1, 128], F16, kind="ExternalInput").ap()
    d["ones1_32"] = nc.dram_tensor("ones1_32", [1, 128], F32, kind="ExternalInput").ap()
    d["bc1"] = nc.dram_tensor("bc1", [1, PS], F16, kind="ExternalInput").ap()
    d["bp0"] = nc.dram_tensor("bp0", [128, 1], F32, kind="ExternalInput").ap()
    d["bkq"] = nc.dram_tensor("bkq", [2, 1], F32, kind="ExternalInput").ap()
    d["alphac"] = nc.dram_tensor("alphac", [128, 1], F32, kind="ExternalInput").ap()

    # per batch: [ssum, r, ssq] concatenated along the free dim
    rss_out = nc.dram_tensor("rss_out", [PB, 3, PS], F32,
                             kind="ExternalOutput").ap()

    QB = SK * PS // 4        # x quarter, free elems (4 k-chunks)
    TB = SK * PS // 3        # x third for the n-major last batch
    WS = CK * PS // 3        # wc1T slab

    with tile.TileContext(nc) as tc:
        with tc.tile_pool(name="const", bufs=1) as cp, \
             tc.tile_pool(name="xp", bufs=2) as xp, \
             tc.tile_pool(name="work", bufs=2) as wp, \
             tc.tile_pool(name="sm", bufs=2) as smp, \
             tc.tile_pool(name="rp", bufs=2) as rp, \
             tc.tile_pool(name="mp0", bufs=1, space="PSUM") as pp0, \
             tc.tile_pool(name="mch", bufs=3, space="PSUM") as pch, \
             tc.tile_pool(name="sml", bufs=2, space="PSUM") as psml, \
             tc.tile_pool(name="auk", bufs=1, space="PSUM") as pakq:

            # ---- DMA issue order fixes the landing schedule:
            # pT, x0 (4 quarters), x1q0, x1q1, wc1s0, x1q2, wc1s1, x1q3,
            # wc1s2, x2 (4 quarters), x3 (3 thirds) ----
            pT_sb = cp.tile([128, SK * NJ], F16, tag="pT")
            nc.sync.dma_start(out=pT_sb[:], in_=d["pT"])

            x_sb = [None] * PB
            wc1_sb = cp.tile([128, CK * PS], F16, tag="wc1")

            def x_tile(b):
                x_sb[b] = xp.tile([128, SK * PS], F16, tag="x",
                                  name=f"x_sb{b}")

            def x_quarter(b, qi):
                nc.sync.dma_start(out=x_sb[b][:, qi * QB:(qi + 1) * QB],
                                  in_=d["xh"][b, :, qi * QB:(qi + 1) * QB])

            def x_third(b, ti):
                nc.sync.dma_start(out=x_sb[b][:, ti * TB:(ti + 1) * TB],
                                  in_=d["xh"][b, :, ti * TB:(ti + 1) * TB])

            def wc1_slab(si):
                nc.sync.dma_start(out=wc1_sb[:, si * WS:(si + 1) * WS],
                                  in_=d["wc1T"][:, si * WS:(si + 1) * WS])

            x_tile(0)
            for qi in range(4):
                x_quarter(0, qi)
            x_tile(1)
            x_quarter(1, 0)
            x_quarter(1, 1)
            wc1_slab(0)
            x_quarter(1, 2)
            wc1_slab(1)
            x_quarter(1, 3)
            wc1_slab(2)
            x_tile(2)
            for qi in range(4):
                x_quarter(2, qi)
            x_tile(3)
            for ti in range(3):
                x_third(3, ti)

            # small constants ride the gpsimd (SWDGE) queue in parallel
            ukq_sb = cp.tile([128, CK * 2], F16, tag="ukq")
            nc.gpsimd.dma_start(out=ukq_sb[:], in_=d["ukq"])
            onesw1_sb = cp.tile([128, 2], F16, tag="onesw1")
            nc.gpsimd.dma_start(out=onesw1_sb[:], in_=d["onesw1"])
            adj_sb = cp.tile([NJ, NJ], F32, tag="adj")
            nc.gpsimd.dma_start(out=adj_sb[:], in_=d["adj"])
            ident_sb = cp.tile([128, 128], F16, tag="ident")
            nc.gpsimd.dma_start(out=ident_sb[:], in_=d["ident"])
            ident2_sb = cp.tile([2, 2], F32, tag="ident2")
            nc.gpsimd.dma_start(out=ident2_sb[:], in_=d["ident2"])
            ones16_sb = cp.tile([1, 128], F16, tag="ones16")
            nc.gpsimd.dma_start(out=ones16_sb[:], in_=d["ones1_16"])
            ones32_sb = cp.tile([1, 128], F32, tag="ones32")
            nc.gpsimd.dma_start(out=ones32_sb[:], in_=d["ones1_32"])
            bc1_sb = cp.tile([1, PS], F16, tag="bc1")
            nc.gpsimd.dma_start(out=bc1_sb[:], in_=d["bc1"])
            bp0_sb = cp.tile([128, 1], F32, tag="bp0")
            nc.gpsimd.dma_start(out=bp0_sb[:], in_=d["bp0"])
            bkq_sb = cp.tile([2, 1], F32, tag="bkq")
            nc.gpsimd.dma_start(out=bkq_sb[:], in_=d["bkq"])
            alpha_sb = cp.tile([128, 1], F32, tag="alphac")
            nc.gpsimd.dma_start(out=alpha_sb[:], in_=d["alphac"])

            # dummy-matmul tile for PE p-state keepalive in window 0
            wu_sb = cp.tile([128, 512], F16, tag="wu")
            nc.vector.memset(wu_sb[:], 0.0)
            wu_ct = [0]

            def emit_wu():
                pw = pch.tile([128, 512], F32, tag="mmt",
                              name=f"wu{wu_ct[0]}")
                wu_ct[0] += 1
                nc.tensor.matmul(pw[:], wu_sb[:, 0:128], wu_sb[:],
                                 start=True, stop=True)

            # ---- per-batch state ----
            hs0T = [[None] * NK for _ in range(PB)]
            hs0 = [None] * PB       # [128, CK*NJ] f16 (c-partition layout)
            hs1T = [None] * PB      # [128, PS] f16
            a1 = [None] * PB        # [NJ, NJ] f16
            kq = [None] * PB
            h2cs = [[None] * NK for _ in range(PB)]
            sqcs = [[None] * NK for _ in range(PB)]
            psAB = [None] * PB      # pool0 k-major psum pair
            psC = [None] * PB       # pool0 pass-B psum
            pcs = [None] * PB       # conv1 psum tiles
            pkq = [None] * PB
            sr_sb = [None] * PB
            ssq_sb = [None] * PB

            # ---------------- emission pieces -------------------------------
            def pool0_stepAB(b, k):
                """One pool0 k-step over column blocks 0,1 (k-major x)."""
                for n in range(2):
                    nc.tensor.matmul(
                        psAB[b][n][:],
                        pT_sb[:, k * NJ:(k + 1) * NJ],
                        x_sb[b][:, k * PS + n * 512: k * PS + n * 512 + 512],
                        start=(k == 0), stop=(k == SK - 1))
                if k == SK - 1:
                    for n in range(2):
                        nc.vector.tensor_scalar_add(
                            hs0T[b][n][:], psAB[b][n][:], bp0_sb[:])

            def passB_group(b, g):
                """Pool0 column block 2, 4 k-steps per thunk (x landed)."""
                if g == 0:
                    psC[b] = pp0.tile([128, 512], F32, tag="mpA",
                                      name=f"pC{b}")
                for k in range(4 * g, 4 * g + 4):
                    nc.tensor.matmul(
                        psC[b][:],
                        pT_sb[:, k * NJ:(k + 1) * NJ],
                        x_sb[b][:, k * PS + 1024: k * PS + 1536],
                        start=(k == 0), stop=(k == SK - 1))
                if g == 3:
                    nc.vector.tensor_scalar_add(
                        hs0T[b][2][:], psC[b][:], bp0_sb[:])

            def pool0_nstep(b, n, k):
                """Last batch (column-block-major x): one (n, k) matmul."""
                if k == 0:
                    psC[b] = pp0.tile([128, 512], F32, tag="mpA",
                                      name=f"pL{b}_{n}")
                nc.tensor.matmul(
                    psC[b][:],
                    pT_sb[:, k * NJ:(k + 1) * NJ],
                    x_sb[b][:, n * (SK * 512) + k * 512:
                             n * (SK * 512) + k * 512 + 512],
                    start=(k == 0), stop=(k == SK - 1))
                if k == SK - 1:
                    nc.vector.tensor_scalar_add(
                        hs0T[b][n][:], psC[b][:], bp0_sb[:])

            def emit_T(b, cc):
                pt = psml.tile([128, 128], F16, tag="sml", name=f"tr{b}_{cc}")
                nc.tensor.transpose(
                    pt[:],
                    hs0T[b][cc // 4][:, (cc % 4) * 128:(cc % 4) * 128 + 128],
                    ident_sb[:])
                nc.vector.tensor_copy(hs0[b][:, cc * NJ:(cc + 1) * NJ], pt[:])

            def emit_kq(b, cc):
                if cc == 0:
                    pkq[b] = pakq.tile([2, 128], F32, tag="kqt",
                                       name=f"pkq{b}")
                nc.tensor.matmul(pkq[b][:], ukq_sb[:, 2 * cc:2 * cc + 2],
                                 hs0[b][:, cc * NJ:(cc + 1) * NJ],
                                 start=(cc == 0), stop=(cc == CK - 1))
                if cc == CK - 1:
                    kq[b] = smp.tile([2, 128], F32, tag="kq", name=f"kq{b}")
                    nc.scalar.activation(kq[b][:], pkq[b][:], AF.Identity,
                                         bias=bkq_sb[:])

            def emit_a1(b):
                """A1 = adj + alpha * tanh(q[j] - k[j']): two PE ops + DVE."""
                pqt = psml.tile([128, 2], F32, tag="sml", name=f"pqt{b}")
                nc.tensor.transpose(pqt[:], kq[b][:], ident2_sb[:])
                qcol_sb = smp.tile([128, 1], F32, tag="qcol", name=f"qcol{b}")
                nc.scalar.activation(qcol_sb[:], pqt[:, 1:2], AF.Copy)
                pbc = psml.tile([128, 128], F32, tag="sml", name=f"pbc{b}")
                nc.tensor.matmul(pbc[:], ones32_sb[:], kq[b][0:1, :],
                                 start=True, stop=True)
                tanh_sb = smp.tile([128, 128], F32, tag="tanh",
                                   name=f"tanh{b}")
                nc.scalar.activation(tanh_sb[:], pbc[:], AF.Tanh,
                                     scale=-1.0, bias=qcol_sb[:])
                a1[b] = smp.tile([NJ, NJ], F16, tag="a1", name=f"a1_{b}")
                nc.vector.tensor_scalar_mul(tanh_sb[:], tanh_sb[:],
                                            alpha_sb[:])
                nc.vector.tensor_add(a1[b][:], tanh_sb[:], adj_sb[:])

            def emit_conv1(b, cc):
                if cc == 0:
                    pcs[b] = [pch.tile([128, 512], F32, tag="mmt",
                                       name=f"c1_{b}_{n}") for n in range(NK)]
                last = cc == CK - 1
                for n in range(NK):
                    nc.tensor.matmul(
                        pcs[b][n][:],
                        hs0[b][:, cc * NJ:(cc + 1) * NJ],
                        wc1_sb[:, cc * PS + n * 512: cc * PS + n * 512 + 512],
                        start=(cc == 0),
                        stop=(not with_bc1 and last))
                if last:
                    if with_bc1:
                        for n in range(NK):
                            nc.tensor.matmul(pcs[b][n][:], ones16_sb[:],
                                             bc1_sb[:, n * 512:(n + 1) * 512],
                                             start=False, stop=True)
                    for n in range(NK):
                        nc.scalar.activation(
                            hs1T[b][:, n * 512:(n + 1) * 512],
                            pcs[b][n][:], AF.Copy)

            def emit_h2(b, n):
                ph = pch.tile([128, 512], F32, tag="mmt", name=f"h2_{b}_{n}")
                nc.tensor.matmul(ph[:], a1[b][:],
                                 hs1T[b][:, n * 512:(n + 1) * 512],
                                 start=True, stop=True)
                h2_sb = wp.tile([128, 512], F16, tag=f"h2c{n}",
                                name=f"h2c{b}_{n}")
                sq_sb = wp.tile([128, 512], F16, tag=f"sqc{n}",
                                name=f"sqc{b}_{n}")
                nc.vector.tensor_copy(h2_sb[:], ph[:])
                nc.scalar.activation(sq_sb[:], ph[:], AF.Square)
                h2cs[b][n] = h2_sb
                sqcs[b][n] = sq_sb

            def emit_stats(b, n):
                if n == 0:
                    sr_sb[b] = rp.tile([2, PS], F32, tag="sr", name=f"sr{b}")
                    ssq_sb[b] = rp.tile([1, PS], F32, tag="ssq",
                                        name=f"ssq{b}")
                sl = slice(n * 512, (n + 1) * 512)
                prs = psml.tile([2, 512], F32, tag="sml", name=f"prs{b}_{n}")
                nc.tensor.matmul(prs[:], onesw1_sb[:], h2cs[b][n][:],
                                 start=True, stop=True)
                nc.scalar.activation(sr_sb[b][:, sl], prs[:], AF.Copy)
                pq2 = psml.tile([1, 512], F32, tag="sml", name=f"pq2{b}_{n}")
                nc.tensor.matmul(pq2[:], onesw1_sb[:, 0:1], sqcs[b][n][:],
                                 start=True, stop=True)
                nc.scalar.activation(ssq_sb[b][:, sl], pq2[:], AF.Copy)
                if n == NK - 1:
                    # last batch rides the HWDGE ring: lower completion
                    # latency on the kernel's critical tail
                    eng = nc.sync if b == PB - 1 else nc.gpsimd
                    eng.dma_start(out=rss_out[b, 0:2, :], in_=sr_sb[b][:])
                    eng.dma_start(out=rss_out[b, 2:3, :], in_=ssq_sb[b][:])

            # ---------------- landing-time model ----------------------------
            pT_B = 128 * SK * NJ * 2
            xq_B = 128 * QB * 2
            xt_B = 128 * TB * 2
            wcs_B = 128 * WS * 2

            land_x = [[0.0] * 4 for _ in range(PB)]
            land_wc = [0.0] * 3
            cum = [pT_B]

            def land(nbytes):
                cum[0] += nbytes
                return DMA_T0 + cum[0] / DMA_BPNS

            for qi in range(4):
                land_x[0][qi] = land(xq_B)
            land_x[1][0] = land(xq_B)
            land_x[1][1] = land(xq_B)
            land_wc[0] = land(wcs_B)
            land_x[1][2] = land(xq_B)
            land_wc[1] = land(wcs_B)
            land_x[1][3] = land(xq_B)
            land_wc[2] = land(wcs_B)
            for qi in range(4):
                land_x[2][qi] = land(xq_B)
            for ti in range(3):
                land_x[3][ti] = land(xt_B)

            # ---------------- thunk lists -----------------------------------
            def tail_thunks(b):
                th = []
                for cc in range(CK):
                    th.append((C_T, 0.0, lambda b=b, cc=cc: emit_T(b, cc)))
                for cc in range(CK):
                    th.append((C_KQ, 0.0, lambda b=b, cc=cc: emit_kq(b, cc)))
                th.append((2 * C_AUX, 0.0, lambda b=b: emit_a1(b)))
                for cc in range(CK):
                    th.append((NK * C_MM512, land_wc[cc // 4] + 300.0,
                               lambda b=b, cc=cc: emit_conv1(b, cc)))
                return th

            def late_thunks(b):
                th = []
                for n in range(NK):
                    th.append((C_MM512, 0.0, lambda b=b, n=n: emit_h2(b, n)))
                for n in range(NK):
                    th.append((2 * C_ST, 0.0,
                               lambda b=b, n=n: emit_stats(b, n)))
                return th

            t_pe = [2000.0]

            def emit_tail(queue, upto):
                while queue and t_pe[0] < upto and queue[0][1] <= t_pe[0]:
                    c, rdy, fn = queue.pop(0)
                    fn()
                    t_pe[0] += c

            # ---------------- main emission loop ----------------------------
            queue = []
            for b in range(PB):
                hs0T[b] = [wp.tile([128, 512], F16, tag=f"hs0T{n}",
                                   name=f"hs0T{b}_{n}") for n in range(NK)]
                hs0[b] = wp.tile([128, CK * NJ], F16, tag="hs0",
                                 name=f"hs0_{b}")
                hs1T[b] = wp.tile([128, PS], F16, tag="hs1T", name=f"hs1T{b}")

                if b >= 2:
                    queue.extend(late_thunks(b - 2))
                if b >= 1:
                    queue.extend(tail_thunks(b - 1))

                if b < PB - 1:
                    psAB[b] = [pp0.tile([128, 512], F32, tag=t,
                                        name=f"p0_{b}_{t}")
                               for t in ("mpA", "mpB")]
                    for k in range(SK):
                        need = land_x[b][k // 4] + 300.0
                        if queue:
                            emit_tail(queue, need)
                        if b == 0:
                            while t_pe[0] < need - C_MM512:
                                emit_wu()
                                t_pe[0] += C_MM512
                        if t_pe[0] < need:
                            t_pe[0] = need
                        pool0_stepAB(b, k)
                        t_pe[0] += 2 * C_MM512
                    # column block 2 runs right after the k-loop: x(b) has
                    # landed, and the next batch's pool0 reuses its PSUM slot
                    for g in range(4):
                        passB_group(b, g)
                        t_pe[0] += 4 * C_MM512
                else:
                    for n in range(NK):
                        for k in range(SK):
                            need = land_x[b][n] + 300.0
                            if queue:
                                emit_tail(queue, need)
                            if t_pe[0] < need:
                                t_pe[0] = need
                            pool0_nstep(b, n, k)
                            t_pe[0] += C_MM512
                        queue.extend(
                            [(C_T, 0.0, lambda b=b, cc=cc: emit_T(b, cc))
                             for cc in range(4 * n, 4 * n + 4)])
                    for cc in range(CK):
                        queue.append(
                            (C_KQ, 0.0, lambda b=b, cc=cc: emit_kq(b, cc)))
                    queue.append((2 * C_AUX, 0.0, lambda b=b: emit_a1(b)))
                    for cc in range(CK):
                        queue.append((NK * C_MM512, 0.0,
                                      lambda b=b, cc=cc: emit_conv1(b, cc)))

            # drain: leftovers, then the last two batches' hs2 + stats
            for c, rdy, fn in queue:
                fn()
            for c, rdy, fn in late_thunks(PB - 2):
                fn()
            for c, rdy, fn in late_thunks(PB - 1):
                fn()

    nc.compile()
    return nc


def _get_nc(with_bc1):
    key = ("nc", with_bc1)
    if key not in _CACHE:
        _CACHE[key] = _build_nc(with_bc1)
    return _CACHE[key]


def kernel(x, w_pool0, b_pool0, adj1, w_q, b_q, w_k, b_k, alpha,
           w_c1, b_c1, gamma, beta, w_pool1, b_pool1, w_cls, b_cls):
    global LAST_EXEC_NS
    x = np.asarray(x, np.float32)

    # ---- host-side input prep (sharding + weight folding) ----
    # (B, S, PS) transpose, then partition-major swizzle: row p holds
    # [xT[k*128+p, :] for k in range(SK)] concatenated.  The last batch of
    # each core instead holds [xT[k*128+p, n*512:(n+1)*512] for n, for k]
    # (column-block-major) so pool0 can run n-outer as thirds land.
    xt = x.reshape(B, PS, S).transpose(0, 2, 1).astype(np.float16)
    xh = np.ascontiguousarray(
        xt.reshape(B, SK, 128, PS).transpose(0, 2, 1, 3)).reshape(
        B, 128, SK * PS)
    pT = np.ascontiguousarray(np.asarray(w_pool0, np.float32).T).astype(np.float16)
    u_q = (np.asarray(w_q, np.float32).sum(0) / QK)
    u_k = (np.asarray(w_k, np.float32).sum(0) / QK)
    ukq = np.stack([u_k, u_q], 1).astype(np.float16)                # (PS, 2)
    wc1T = np.ascontiguousarray(np.asarray(w_c1, np.float32).T).astype(np.float16)
    onesw1 = np.stack([np.ones(NJ, np.float32),
                       np.asarray(w_pool1, np.float32)[0]], 1).astype(np.float16)

    common = {
        "pT": np.ascontiguousarray(
            pT.reshape(SK, 128, NJ).transpose(1, 0, 2)).reshape(128, SK * NJ),
        "wc1T": np.ascontiguousarray(
            wc1T.reshape(CK, 128, PS).transpose(1, 0, 2)).reshape(128, CK * PS),
        "ukq": np.ascontiguousarray(
            ukq.reshape(CK, 128, 2).transpose(1, 0, 2)).reshape(128, CK * 2),
        "onesw1": onesw1,
        "adj": np.asarray(adj1, np.float32),
        "ident": np.eye(128, dtype=np.float16),
        "ident2": np.eye(2, dtype=np.float32),
        "ones1_16": np.ones((1, 128), np.float16),
        "ones1_32": np.ones((1, 128), np.float32),
        "bc1": np.asarray(b_c1, np.float32)[None, :].astype(np.float16),
        "bp0": np.asarray(b_pool0, np.float32)[:, None],
        "bkq": np.array([[np.asarray(b_k, np.float32).mean()],
                         [np.asarray(b_q, np.float32).mean()]], np.float32),
        "alphac": np.full((128, 1), np.asarray(alpha, np.float32)[0], np.float32),
    }
    # last batch of each core: column-block-major swizzle
    xh_nmaj = np.ascontiguousarray(
        xt.reshape(B, SK, 128, NK, 512).transpose(0, 2, 3, 1, 4)).reshape(
        B, 128, SK * PS)
    in_maps = []
    for c in range(NCORES):
        m = dict(common)
        xs = np.empty((PB, 128, SK * PS), np.float16)
        xs[:PB - 1] = xh[c * PB:c * PB + PB - 1]
        xs[PB - 1] = xh_nmaj[c * PB + PB - 1]
        m["xh"] = np.ascontiguousarray(xs)
        in_maps.append(m)

    nc = _get_nc(bool(np.any(np.asarray(b_c1))))
    res = run_bass_kernel_spmd(nc, in_maps, list(range(NCORES)), trace=TRACE,
                               tmpdir=TMPDIR)
    LAST_EXEC_NS = res.exec_time_ns

    # ---- host epilogue: BN stats all-reduce + affine + classifier ----
    rss = np.stack([res.results[c]["rss_out"] for c in range(NCORES)])
    ssum = rss[:, :, 0, :].sum((0, 1)).astype(np.float64)
    r_all = rss[:, :, 1, :].reshape(B, PS)
    ssq = rss[:, :, 2, :].sum((0, 1)).astype(np.float64)
    n = B * NJ
    mean = ssum / n
    var = ssq / n - mean * mean
    s = np.asarray(gamma, np.float64) / np.sqrt(var + BN_EPS)
    t = np.asarray(beta, np.float64) - s * mean
    w1sum = float(np.asarray(w_pool1, np.float64)[0].sum())
    pooled = s[None, :] * r_all.astype(np.float64) \
        + (t * w1sum + float(np.asarray(b_pool1)[0]))[None, :]
    out = pooled @ np.asarray(w_cls, np.float64).T + np.asarray(b_cls, np.float64)
    return out.astype(np.float32)
